# revision 15
# baseline (speedup 1.0000x reference)
"""GNN message-passing kernel for Trainium2 (8 NeuronCores, SPMD).

Reference computation:
    msg = x_feat[src]                                # [E, H] gather
    xee = gelu(msg @ w_pre + b_pre) * bases          # [E, H]
    y   = segment_sum(xee, dst, N)                   # [N, H]
    x   = x_feat + y
    h   = relu(bn1(x @ w1 + b1)); h = relu(bn2(h @ w2 + b2))
    out = x + h

Sharding: nodes are partitioned into 8 contiguous ranges (graph parallel,
dst-range sharding).  All indices (src, dst) are known at build time, so the
host pre-permutes x_feat rows into dst-sorted edge-slot order (a pure data
permutation; all arithmetic stays on device).  Each core then:
  1. streams its slot-ordered gathered features xg [f, slot] and computes
     xee = gelu(xg @ w_pre + b_pre) per edge on TensorE (+ScalarE),
  2. multiplies by the slot-ordered bases shard on VectorE,
  3. scatter-sums into 128-destination-node PSUM windows via one-hot
     matmuls on TensorE (one-hot built per tile on VectorE with a 4x-mode
     tensor_scalar is_equal against a resident iota),
  4. adds the x residual; the dense FFN (+folded BN) runs inline as soon
     as each 512-column group of windows closes, hiding it behind the
     message phase.
The chunk loop is software-pipelined one deep (pre-GEMM of chunk k+1 is
emitted before the scatter of chunk k) so TensorE never waits on VectorE.
DMA issue is spread over the sync (xg), gpsimd (bases) and vector (x
residual) queues.  Host re-assembles the 8 output slices.
"""

import os
from contextlib import ExitStack

import numpy as np

import concourse.bass as bass
import concourse.tile as tile
from concourse import bacc, mybir
from concourse.bass_utils import run_bass_kernel_spmd

F32 = mybir.dt.float32
F16 = mybir.dt.float16
U8 = mybir.dt.uint8
NP_F16 = np.float16

H = 128          # hidden dim == partition count
N_CORES = 8
GCHUNK = int(os.environ.get("GNN_GCHUNK", "32"))   # edge tiles per chunk
WIN = int(os.environ.get("GNN_WIN", "64"))         # dst-window width (nodes)

LAST_RESULTS = None    # stashed BassKernelResults from the most recent run


# ---------------------------------------------------------------------------
# host-side sharding / edge preprocessing (index permutation only)
# ---------------------------------------------------------------------------

def _ceil_div(a, b):
    return (a + b - 1) // b


def _preprocess(x_feat, bases, src, dst, n_cores):
    """Shard edges by destination-node range; per core build dst-sorted,
    window-aligned slot arrays: gathered-x (feature-major), bases
    (edge-major), and local-dst labels."""
    n_nodes = x_feat.shape[0]
    assert n_nodes % n_cores == 0
    ns = n_nodes // n_cores                 # nodes per shard
    n_win = _ceil_div(ns, WIN)              # WIN-node windows per shard

    per = [[None] * n_win for _ in range(n_cores)]
    g = np.zeros(n_win, np.int64)           # tiles per window (max over cores)
    for c in range(n_cores):
        lo_node = c * ns
        sel = np.nonzero((dst >= lo_node) & (dst < lo_node + ns))[0]
        d = dst[sel] - lo_node
        order = np.argsort(d, kind="stable")
        eid = sel[order]
        d = d[order]
        w_of = d // WIN
        bounds = np.searchsorted(w_of, np.arange(n_win + 1))
        for w in range(n_win):
            a, b = bounds[w], bounds[w + 1]
            per[c][w] = (eid[a:b], d[a:b] - w * WIN)
            g[w] = max(g[w], _ceil_div(b - a, 128))

    n_tiles = int(g.sum())
    nslot = n_tiles * 128
    offs = np.concatenate([[0], np.cumsum(g[:-1])]) * 128

    x16 = x_feat.astype(NP_F16)
    b16 = bases.astype(NP_F16)
    cores = []
    for c in range(n_cores):
        slot_edge = np.full(nslot, -1, np.int64)
        dstl = np.full(nslot, -1.0, np.float32)
        for w in range(n_win):
            eid, dl = per[c][w]
            k = len(eid)
            slot_edge[offs[w]:offs[w] + k] = eid
            dstl[offs[w]:offs[w] + k] = dl
        m = slot_edge >= 0
        # gathered x rows, feature-major: xg[f, slot]
        xg_slot = np.zeros((nslot, H), NP_F16)
        xg_slot[m] = x16[src[slot_edge[m]]]
        xg = np.ascontiguousarray(xg_slot.T)
        # bases rows, edge-in-tile-major: bs2[e, t*128 + f]
        bs_slot = np.zeros((nslot, H), NP_F16)
        bs_slot[m] = b16[slot_edge[m]]
        bs2 = np.ascontiguousarray(
            bs_slot.reshape(n_tiles, 128, H).transpose(1, 0, 2)
            .reshape(128, n_tiles * H))
        # local dst labels: dstl2[e, t]
        dstl2 = np.ascontiguousarray(
            dstl.reshape(n_tiles, 128).T.astype(NP_F16))
        cores.append(dict(xg=xg, bs2=bs2, dstl=dstl2))
    return ns, n_win, g, n_tiles, cores


# ---------------------------------------------------------------------------
# device program
# ---------------------------------------------------------------------------

def build_program(tc, cfg, io):
    """Emit the per-core Tile program.  io maps names -> DRAM APs."""
    nc = tc.nc
    ctx = ExitStack()
    ns = cfg["ns"]
    n_win = cfg["n_win"]
    g = cfg["g"]
    n_tiles = cfg["n_tiles"]
    has_bpre = cfg["has_bpre"]

    with ctx:
        consts = ctx.enter_context(tc.tile_pool(name="consts", bufs=1))
        xg_pool = ctx.enter_context(tc.tile_pool(name="xg", bufs=3))
        b_pool = ctx.enter_context(tc.tile_pool(name="base", bufs=3))
        e_pool = ctx.enter_context(tc.tile_pool(name="xee", bufs=3))
        p_pool = ctx.enter_context(tc.tile_pool(name="phot", bufs=2))
        g_psum = ctx.enter_context(tc.tile_pool(name="gpsum", bufs=2, space="PSUM"))
        s_psum = ctx.enter_context(tc.tile_pool(name="spsum", bufs=2, space="PSUM"))
        f_psum = ctx.enter_context(tc.tile_pool(name="fpsum", bufs=1, space="PSUM"))
        acc_pool = ctx.enter_context(tc.tile_pool(name="acc", bufs=1))
        ffn_pool = ctx.enter_context(tc.tile_pool(name="ffn", bufs=2))

        # constants packed into two blob DMAs (one f16, one f32) so the
        # sync queue reaches chunk-0 data with minimal per-DMA latency
        nw16 = 5 if has_bpre else 3
        cf16 = consts.tile([H, nw16 * H], F16)
        nc.sync.dma_start(cf16[:], io["cb16"])
        cf32 = consts.tile([H, 4], F32)
        nc.sync.dma_start(cf32[:], io["cb32"])
        w_pre_sb = cf16[:, 0:H]
        w1_sb = cf16[:, H:2 * H]
        w2_sb = cf16[:, 2 * H:3 * H]
        if has_bpre:
            ones_sb = cf16[0:1, 3 * H:4 * H]
            bpre_sb = cf16[0:1, 4 * H:5 * H]
        s1_sb = cf32[:, 0:1]
        o1_sb = cf32[:, 1:2]
        s2_sb = cf32[:, 2:3]
        o2_sb = cf32[:, 3:4]
        iota_sb = consts.tile([H, GCHUNK * WIN], F16)
        nc.gpsimd.dma_start(iota_sb[:], io["iota"])
        dstl_sb = consts.tile([H, n_tiles], F16)
        nc.gpsimd.dma_start(dstl_sb[:], io["dstl"])

        # ---- scatter accumulator (x residual is added in the FFN phase,
        # so no big blocking DMA sits ahead of the edge stream) ----
        yT = acc_pool.tile([H, ns], F32)
        nc.vector.memset(yT[:], 0.0)
        xs_sb = acc_pool.tile([H, ns], F16)
        out_sb = acc_pool.tile([H, ns], F16)

        # tile -> (window, first, last)
        tmeta = []
        for w in range(n_win):
            for j in range(g[w]):
                tmeta.append((w, j == 0, j == g[w] - 1))

        # chunk schedule: small chunks first (fast pipeline ramp), then GCHUNK
        sched = []
        pos = 0
        for size in (8, 8, 16, 16):
            if pos + size <= n_tiles:
                sched.append((pos, size))
                pos += size
        tail = [16, 8, 8]
        tail_start = n_tiles - sum(tail)
        while pos < tail_start:
            ct = min(GCHUNK, tail_start - pos)
            sched.append((pos, ct))
            pos += ct
        for size in tail:
            if pos < n_tiles:
                ct = min(size, n_tiles - pos)
                sched.append((pos, ct))
                pos += ct
        n_chunk = len(sched)
        chunks = {}          # live per-chunk tiles
        psum_cur = [None]
        ffn_pending = []

        def stage_load(k):
            """DMA chunk k's xg (sync queue) and bases (gpsimd queue)."""
            t0, ct = sched[k]
            xg = xg_pool.tile([H, GCHUNK * H], F16)
            nc.sync.dma_start(xg[:, :ct * H],
                              io["xg"][:, t0 * H:(t0 + ct) * H])
            bs = b_pool.tile([H, GCHUNK * H], F16)
            nc.gpsimd.dma_start(bs[:, :ct * H],
                                io["bs2"][:, t0 * H:(t0 + ct) * H])
            chunks[k] = dict(xg=xg, bs=bs, ct=ct, t0=t0)

        def stage_pre(k):
            """Pre-GEMM + gelu for chunk k (TensorE + ScalarE)."""
            c = chunks[k]
            xg, ct = c["xg"], c["ct"]
            xe = e_pool.tile([H, GCHUNK * H], F16)
            for q0 in range(0, ct, 8):
                qn = min(8, ct - q0)
                p1 = g_psum.tile([H, 1024], F32, space="PSUM")
                for j in range(qn):
                    nc.tensor.matmul(
                        p1[:, j * H:(j + 1) * H],
                        lhsT=xg[:, (q0 + j) * H:(q0 + j + 1) * H],
                        rhs=w_pre_sb[:],
                        start=True, stop=not has_bpre)
                    if has_bpre:
                        nc.tensor.matmul(
                            p1[:, j * H:(j + 1) * H],
                            lhsT=ones_sb,
                            rhs=bpre_sb,
                            start=False, stop=True)
                nc.scalar.activation(
                    xe[:, q0 * H:(q0 + qn) * H], p1[:, :qn * H],
                    mybir.ActivationFunctionType.Gelu)
            c["xe"] = xe

        def ffn_group(m):
            """FFN + BN + residual for the 512-col group m (windows 4m..)."""
            k0 = m * 512
            kn = min(512, ns - k0)
            y16 = ffn_pool.tile([H, 512], F16, name=f"y16_{m}", tag="y16")
            nc.vector.tensor_tensor(
                out=y16[:, :kn], in0=yT[:, k0:k0 + kn],
                in1=xs_sb[:, k0:k0 + kn], op=mybir.AluOpType.add)
            h1p = f_psum.tile([H, 512], F32, space="PSUM")
            nc.tensor.matmul(h1p[:, :kn], lhsT=w1_sb, rhs=y16[:, :kn],
                             start=True, stop=True)
            h1 = ffn_pool.tile([H, 512], F16, name=f"h1_{m}", tag="h1")
            nc.scalar.activation(h1[:, :kn], h1p[:, :kn],
                                 mybir.ActivationFunctionType.Relu,
                                 bias=o1_sb, scale=s1_sb)
            h2p = f_psum.tile([H, 512], F32, space="PSUM")
            nc.tensor.matmul(h2p[:, :kn], lhsT=w2_sb, rhs=h1[:, :kn],
                             start=True, stop=True)
            h2 = ffn_pool.tile([H, 512], F32, name=f"h2_{m}", tag="h2")
            nc.scalar.activation(h2[:, :kn], h2p[:, :kn],
                                 mybir.ActivationFunctionType.Relu,
                                 bias=o2_sb, scale=s2_sb)
            nc.vector.tensor_add(out=out_sb[:, k0:k0 + kn],
                                 in0=y16[:, :kn], in1=h2[:, :kn])
            nc.sync.dma_start(io["y_out"][:, k0:k0 + kn],
                              out_sb[:, k0:k0 + kn])

        def stage_scatter(k):
            """bases multiply, one-hot gen, scatter matmuls for chunk k.
            FFN groups whose windows closed in earlier chunks are emitted
            first -- their inputs are long since ready, so they slot into
            engine idle time instead of head-of-line-blocking the PE queue."""
            while ffn_pending:
                ffn_group(ffn_pending.pop(0))
            c = chunks.pop(k)
            t0 = c["t0"]
            xe, bs, ct = c["xe"], c["bs"], c["ct"]
            nc.vector.tensor_tensor(
                out=xe[:, :ct * H], in0=xe[:, :ct * H], in1=bs[:, :ct * H],
                op=mybir.AluOpType.mult)
            pc = p_pool.tile([H, GCHUNK * WIN], F16)
            nc.vector.tensor_tensor(
                out=pc[:, :ct * WIN].rearrange("p (t f) -> p t f", f=WIN),
                in0=dstl_sb[:, t0:t0 + ct].to_broadcast([H, ct, WIN]),
                in1=iota_sb[:, :ct * WIN].rearrange("p (t f) -> p t f", f=WIN),
                op=mybir.AluOpType.is_equal)
            for j in range(ct):
                t = t0 + j
                w, first, last = tmeta[t]
                if first:
                    psum_cur[0] = s_psum.tile([H, WIN], F32, space="PSUM",
                                              name=f"win{w}", tag="win_ps")
                nc.tensor.matmul(
                    psum_cur[0][:],
                    lhsT=xe[:, j * H:(j + 1) * H],
                    rhs=pc[:, j * WIN:(j + 1) * WIN],
                    start=first, stop=last)
                if last:
                    wn = min(WIN, ns - w * WIN)
                    nc.vector.tensor_add(
                        out=yT[:, w * WIN:w * WIN + wn],
                        in0=yT[:, w * WIN:w * WIN + wn],
                        in1=psum_cur[0][:, :wn])
                    wpg = 512 // WIN
                    if (w + 1) % wpg == 0 or w == n_win - 1:
                        ffn_pending.append(w // wpg)

        # software-pipelined main loop
        stage_load(0)
        stage_pre(0)
        nc.scalar.dma_start(xs_sb[:], io["xs16"])
        for k in range(n_chunk):
            if k + 1 < n_chunk:
                stage_load(k + 1)
                stage_pre(k + 1)
            stage_scatter(k)
        while ffn_pending:
            ffn_group(ffn_pending.pop(0))


# ---------------------------------------------------------------------------
# top level
# ---------------------------------------------------------------------------

def _fold_bn(g, be, m, v, b, eps=1e-5):
    s = g / np.sqrt(v + eps)
    o = be - m * s + b * s
    return s.astype(np.float32).reshape(H, 1), o.astype(np.float32).reshape(H, 1)


def build(x_feat, bases, src, dst, w_pre, b_pre,
          w1, b1, g1, be1, m1, v1,
          w2, b2, g2, be2, m2, v2,
          n_cores=N_CORES):
    """Build the compiled Bass module + per-core input maps."""
    x_feat = np.asarray(x_feat, np.float32)
    bases = np.asarray(bases, np.float32)
    src = np.asarray(src, np.int64)
    dst = np.asarray(dst, np.int64)
    w_pre = np.asarray(w_pre, np.float32)
    b_pre = np.asarray(b_pre, np.float32)
    w1 = np.asarray(w1, np.float32)
    w2 = np.asarray(w2, np.float32)

    n_nodes = x_feat.shape[0]
    ns, n_win, g, n_tiles, cores = _preprocess(
        x_feat, bases, src, dst, n_cores)
    has_bpre = bool(np.any(b_pre != 0))

    s1, o1 = _fold_bn(np.asarray(g1, np.float32), np.asarray(be1, np.float32),
                      np.asarray(m1, np.float32), np.asarray(v1, np.float32),
                      np.asarray(b1, np.float32))
    s2, o2 = _fold_bn(np.asarray(g2, np.float32), np.asarray(be2, np.float32),
                      np.asarray(m2, np.float32), np.asarray(v2, np.float32),
                      np.asarray(b2, np.float32))
    iota = np.ascontiguousarray(
        np.tile(np.arange(WIN, dtype=NP_F16), (H, GCHUNK)))

    cfg = dict(ns=ns, n_win=n_win, g=list(g), n_tiles=n_tiles,
               has_bpre=has_bpre)

    nc = bacc.Bacc("TRN2", target_bir_lowering=False, debug=False,
                   num_devices=n_cores)

    nw16 = 5 if has_bpre else 3
    shapes = dict(
        xs16=(H, ns),
        cb16=(H, nw16 * H), cb32=(H, 4),
        iota=(H, GCHUNK * WIN),
        xg=(H, n_tiles * H), bs2=(H, n_tiles * H), dstl=(H, n_tiles),
    )
    dts = {n: F16 for n in ("xg", "bs2", "dstl", "iota", "xs16", "cb16")}

    io = {}
    for name, shp in shapes.items():
        io[name] = nc.dram_tensor(name, list(shp), dts.get(name, F32),
                                  kind="ExternalInput").ap()
    io["y_out"] = nc.dram_tensor("y_out", [H, ns], F16,
                                 kind="ExternalOutput").ap()

    with tile.TileContext(nc) as tc:
        build_program(tc, cfg, io)

    cb16 = [w_pre.astype(NP_F16), w1.astype(NP_F16), w2.astype(NP_F16)]
    if has_bpre:
        cb16.append(np.ones((H, H), NP_F16))          # row 0 used
        cb16.append(np.tile(b_pre.reshape(1, H), (H, 1)).astype(NP_F16))
    cb16 = np.ascontiguousarray(np.concatenate(cb16, axis=1))
    cb32 = np.ascontiguousarray(np.concatenate([s1, o1, s2, o2], axis=1))
    shared = dict(cb16=cb16, cb32=cb32, iota=iota)

    in_maps = []
    xT = x_feat.T
    for c in range(n_cores):
        m = dict(shared)
        m["xs16"] = np.ascontiguousarray(
            xT[:, c * ns:(c + 1) * ns]).astype(NP_F16)
        for k in ("xg", "bs2", "dstl"):
            m[k] = cores[c][k]
        in_maps.append(m)

    nc.compile()
    return nc, in_maps, ns, n_nodes


def kernel(trace=False, n_cores=N_CORES, **inputs):
    global LAST_RESULTS
    nc, in_maps, ns, n_nodes = build(n_cores=n_cores, **inputs)
    res = run_bass_kernel_spmd(
        nc, in_maps, core_ids=list(range(n_cores)), trace=trace,
        trace_cores=list(range(n_cores)) if trace else None)
    LAST_RESULTS = res

    out = np.empty((n_nodes, H), np.float32)
    for c in range(n_cores):
        out[c * ns:(c + 1) * ns] = res.results[c]["y_out"].astype(np.float32).T
    return out
